# revision 3
# baseline (speedup 1.0000x reference)
"""Trainium2 Bass kernel for nn_Block_627065225827 (dense_transformer).

Self-contained: hardcodes shapes B=32, T=4096, C=256, H=8 and the
data-parallel-over-batch sharding (4 batch rows per core, 8 cores).

Math (see reference):
    h   = LN1(x) * g1 + b1ln
    id  = h @ w_id.T ;  inf = h @ w_inf.T            (per-head view [H, hs])
    inf = inf / (1+K);  shifted[t] = inf[t - s_h]    (zero for t < s_h)
    sa  = (K/(1+K) * id + shifted) @ w_proj.T + b_proj
    x1  = x + sa
    ff  = relu(LN2(x1)*g2+b2ln @ w1.T + b1) @ w2.T + b2
    out = x1 + ff

Host-side algebraic folding (exact):
    sa[t] = w_a @ xhat[t] + sum_s W_s @ xhat[t-s] + const_rows
      w_a = w_proj @ (diag(a_row) @ (w_id * g1))           a_h = K/(1+K)
      W_s = w_proj[:, cols_s] @ ((w_inf * g1) * binv)[cols_s, :]
    so the per-head temporal shift becomes a free-dim offset into the
    transposed activation buffer hB (channels on partitions, tokens on
    free dim), with 4 zero pad columns implementing the t<s mask.
"""

import os
from contextlib import ExitStack

import numpy as np
import ml_dtypes

B, T, C, H = 32, 4096, 256, 8
HS = C // H
NCORES = 8
BPC = B // NCORES  # batch rows per core
SHIFTS = [1, 2, 3, 4, 1, 2, 3, 4]
EPS = 1e-5
PAD = 4  # zero columns at the head of hB for the shift mask
WIN = 512  # tokens per window
SUB = 128  # tokens per subtile (partition dim)

_f64 = np.float64
_bf16 = ml_dtypes.bfloat16


def _prep(inputs):
    """Fold LN gains/biases + per-head scalars into the weights (host, numpy)."""
    g = {k: np.asarray(v, dtype=_f64) for k, v in inputs.items() if k != "x"}
    K = np.exp(g["khead"])  # [H]
    a_row = np.repeat(K / (1.0 + K), HS)  # [C] per id-output channel
    b_row = np.repeat(1.0 / (1.0 + K), HS)  # [C] per inf-output channel

    w_id_g = g["w_id"] * g["ln1_g"][None, :]
    w_inf_g = g["w_inf"] * g["ln1_g"][None, :]
    w_id_s = w_id_g * a_row[:, None]
    w_inf_s = w_inf_g * b_row[:, None]

    w_a = g["w_proj"] @ w_id_s  # [C, C]
    wsT = np.zeros((4, C, C), _f64)
    c_s = np.zeros((4, C), _f64)
    cid = w_id_g @ g["ln1_b"]
    cinf = w_inf_g @ g["ln1_b"]
    for s in range(1, 5):
        cols = np.concatenate(
            [np.arange(h * HS, (h + 1) * HS) for h in range(H) if SHIFTS[h] == s]
        )
        wsT[s - 1] = (g["w_proj"][:, cols] @ w_inf_s[cols, :]).T
        c_s[s - 1] = g["w_proj"][:, cols] @ (b_row * cinf)[cols]
    c_a = g["w_proj"] @ (a_row * cid) + g["b_proj"]  # [C]

    w1_g = g["w1"] * g["ln2_g"][None, :]
    b1_eff = g["w1"] @ g["ln2_b"] + g["b1"]  # [4C]
    w2T = g["w2"].T  # [4C, C]
    b2_eff = g["b2"]  # [C]

    out = {
        "w_aT": np.ascontiguousarray(w_a.T.reshape(2, 128, C)).astype(_bf16),
        "wsT": np.ascontiguousarray(wsT.reshape(4, 2, 128, C)).astype(_bf16),
        "w1T": np.ascontiguousarray(w1_g.T.reshape(2, 128, 4 * C)).astype(_bf16),
        "w2T": np.ascontiguousarray(w2T.reshape(8, 128, C)).astype(_bf16),
        "b1c": np.ascontiguousarray(b1_eff.reshape(8, 128).T).astype(np.float32),
    }
    crows_attn = np.stack([c_a, c_s[0], c_s[1], c_s[2], c_s[3]])  # [5, C]
    use_crows = bool(np.abs(crows_attn).max() > 0)
    use_b2row = bool(np.abs(b2_eff).max() > 0)
    if use_crows or use_b2row:
        out["crows"] = np.concatenate([crows_attn, b2_eff[None, :]]).reshape(1, 6 * C).astype(_bf16)
    return out, use_crows, use_b2row


def _build(n_rows=BPC, t_len=T, use_crows=False, use_b2row=False):
    """Build + compile the per-core Bass program. Returns the finalized nc."""
    import concourse.bacc as bacc
    import concourse.mybir as mybir
    import concourse.tile as tile
    from concourse.masks import make_identity

    dt = mybir.dt
    AF = mybir.ActivationFunctionType
    OP = mybir.AluOpType

    nwin = t_len // WIN
    nc = bacc.Bacc("TRN2", target_bir_lowering=False, debug=False, num_devices=NCORES)

    x_d = nc.declare_dram_parameter("x", [n_rows, t_len, C], dt.float32, isOutput=False)
    waT_d = nc.declare_dram_parameter("w_aT", [2, 128, C], dt.bfloat16, isOutput=False)
    wsT_d = nc.declare_dram_parameter("wsT", [4, 2, 128, C], dt.bfloat16, isOutput=False)
    w1T_d = nc.declare_dram_parameter("w1T", [2, 128, 4 * C], dt.bfloat16, isOutput=False)
    w2T_d = nc.declare_dram_parameter("w2T", [8, 128, C], dt.bfloat16, isOutput=False)
    b1c_d = nc.declare_dram_parameter("b1c", [128, 8], dt.float32, isOutput=False)
    crows_d = None
    if use_crows or use_b2row:
        crows_d = nc.declare_dram_parameter("crows", [1, 6 * C], dt.bfloat16, isOutput=False)
    out_d = nc.declare_dram_parameter("out", [n_rows, t_len, C], dt.float32, isOutput=True)

    with tile.TileContext(nc) as tc, ExitStack() as ctx:
        singles = ctx.enter_context(tc.tile_pool(name="singles", bufs=1))
        hb_pool = ctx.enter_context(tc.tile_pool(name="hb", bufs=1))
        xin = ctx.enter_context(tc.tile_pool(name="xin", bufs=6))
        x1p = ctx.enter_context(tc.tile_pool(name="x1p", bufs=6))
        hnorm = ctx.enter_context(tc.tile_pool(name="hnorm", bufs=4))
        stats = ctx.enter_context(tc.tile_pool(name="stats", bufs=8))
        h2b = ctx.enter_context(tc.tile_pool(name="h2b", bufs=2))
        ffb = ctx.enter_context(tc.tile_pool(name="ffb", bufs=2))
        outp = ctx.enter_context(tc.tile_pool(name="outp", bufs=4))
        tp_ps = ctx.enter_context(tc.tile_pool(name="tp_ps", bufs=2, space="PSUM"))
        sa_ps = ctx.enter_context(tc.tile_pool(name="sa_ps", bufs=2, space="PSUM"))
        up_ps = ctx.enter_context(tc.tile_pool(name="up_ps", bufs=2, space="PSUM"))
        dn_ps = ctx.enter_context(tc.tile_pool(name="dn_ps", bufs=2, space="PSUM"))

        # ---- constants / weights in SBUF ----
        ident = singles.tile([128, 128], dt.bfloat16)
        make_identity(nc, ident)
        eps_t = singles.tile([128, 1], dt.float32)
        nc.vector.memset(eps_t, EPS)
        waT = []
        for c in range(2):
            w = singles.tile([128, C], dt.bfloat16, tag=f"waT{c}")
            nc.gpsimd.dma_start(out=w, in_=waT_d[c])
            waT.append(w)
        wsT = []
        for s in range(4):
            row = []
            for c in range(2):
                w = singles.tile([128, C], dt.bfloat16, tag=f"wsT{s}{c}")
                nc.gpsimd.dma_start(out=w, in_=wsT_d[s, c])
                row.append(w)
            wsT.append(row)
        w1T = []
        for c in range(2):
            w = singles.tile([128, 4 * C], dt.bfloat16, tag=f"w1T{c}")
            nc.gpsimd.dma_start(out=w, in_=w1T_d[c])
            w1T.append(w)
        w2T = []
        for fc in range(8):
            w = singles.tile([128, C], dt.bfloat16, tag=f"w2T{fc}")
            nc.gpsimd.dma_start(out=w, in_=w2T_d[fc])
            w2T.append(w)
        b1c = singles.tile([128, 8], dt.float32)
        nc.gpsimd.dma_start(out=b1c, in_=b1c_d[:, :])
        crows = None
        ones_row = None
        if crows_d is not None:
            crows = singles.tile([1, 6 * C], dt.bfloat16)
            nc.gpsimd.dma_start(out=crows, in_=crows_d[:, :])
            ones_row = singles.tile([1, PAD + t_len], dt.bfloat16)
            nc.vector.memset(ones_row, 1.0)
            nc.vector.memset(ones_row[:, 0:PAD], 0.0)

        def layernorm_to(src, dst0, dst1):
            """LN(src [128,C] f32) -> bf16, transposed into dst0/dst1 [128,128]."""
            st = stats.tile([128, 6], dt.float32, tag="st")
            nc.vector.bn_stats(st, src)
            mv = stats.tile([128, 2], dt.float32, tag="mv")
            nc.vector.bn_aggr(mv, st)
            sd = stats.tile([128, 1], dt.float32, tag="sd")
            nc.scalar.activation(sd, mv[:, 1:2], AF.Sqrt, bias=eps_t, scale=1.0)
            rs = stats.tile([128, 1], dt.float32, tag="rs")
            nc.vector.reciprocal(rs, sd)
            hn = hnorm.tile([128, C], dt.bfloat16, tag="hn")
            nc.vector.tensor_scalar(
                out=hn, in0=src, scalar1=mv[:, 0:1], scalar2=rs,
                op0=OP.subtract, op1=OP.mult,
            )
            for c, dst in enumerate((dst0, dst1)):
                pt = tp_ps.tile([128, 128], dt.bfloat16, tag="tp")
                nc.tensor.transpose(pt, hn[:, 128 * c:128 * (c + 1)], ident)
                if c == 0:
                    nc.vector.tensor_copy(out=dst, in_=pt)
                else:
                    nc.scalar.copy(out=dst, in_=pt)

        for r in range(n_rows):
            hB = []
            for c in range(2):
                t_ = hb_pool.tile([128, PAD + t_len], dt.bfloat16, tag=f"hb{c}")
                nc.gpsimd.memset(t_[:, 0:PAD], 0.0)
                hB.append(t_)

            for w in range(nwin):
                t0w = w * WIN
                x_tiles, x1_tiles = [], []
                h2 = [h2b.tile([128, WIN], dt.bfloat16, tag=f"h2b{c}", name=f"h2b{c}") for c in range(2)]
                for k in range(4):
                    t0 = t0w + k * SUB
                    col = PAD + t0
                    xt = xin.tile([128, C], dt.float32, tag="x")
                    nc.sync.dma_start(out=xt, in_=x_d[r, t0:t0 + SUB, :])
                    x_tiles.append(xt)
                    # LN1 -> hB columns
                    layernorm_to(xt, hB[0][:, col:col + SUB], hB[1][:, col:col + SUB])
                    # attention: sa[t, co] accumulated over 10 matmuls
                    ps = sa_ps.tile([128, C], dt.float32, tag="sa")
                    nc.tensor.matmul(ps, hB[0][:, col:col + SUB], waT[0],
                                     start=True, stop=False)
                    nc.tensor.matmul(ps, hB[1][:, col:col + SUB], waT[1],
                                     start=False, stop=False)
                    for s in range(1, 5):
                        for c in range(2):
                            last = (s == 4 and c == 1 and not use_crows)
                            nc.tensor.matmul(
                                ps, hB[c][:, col - s:col - s + SUB], wsT[s - 1][c],
                                start=False, stop=last,
                            )
                    if use_crows:
                        nc.tensor.matmul(ps, ones_row[:, col:col + SUB], crows[:, 0:C],
                                         start=False, stop=False)
                        for s in range(1, 5):
                            nc.tensor.matmul(
                                ps, ones_row[:, col - s:col - s + SUB],
                                crows[:, s * C:(s + 1) * C], start=False, stop=(s == 4),
                            )
                    x1t = x1p.tile([128, C], dt.float32, tag="x1")
                    nc.vector.tensor_add(out=x1t, in0=xt, in1=ps)
                    x1_tiles.append(x1t)
                    # LN2 -> h2 window columns
                    layernorm_to(x1t, h2[0][:, k * SUB:(k + 1) * SUB],
                                 h2[1][:, k * SUB:(k + 1) * SUB])

                # FFN up + relu (weights stationary, tokens moving)
                fftiles = []
                for fc in range(8):
                    pu = up_ps.tile([128, WIN], dt.float32, tag="up")
                    nc.tensor.matmul(pu, w1T[0][:, 128 * fc:128 * (fc + 1)], h2[0],
                                     start=True, stop=False)
                    nc.tensor.matmul(pu, w1T[1][:, 128 * fc:128 * (fc + 1)], h2[1],
                                     start=False, stop=True)
                    fb = ffb.tile([128, WIN], dt.bfloat16, tag=f"ffb{fc}")
                    nc.scalar.activation(fb, pu, AF.Relu, bias=b1c[:, fc:fc + 1],
                                         scale=1.0)
                    fftiles.append(fb)

                # FFN down (ff stationary, w2T moving) + residual + store
                for k in range(4):
                    t0 = t0w + k * SUB
                    pd = dn_ps.tile([128, C], dt.float32, tag="dn")
                    for fc in range(8):
                        last = (fc == 7 and not use_b2row)
                        nc.tensor.matmul(pd, fftiles[fc][:, k * SUB:(k + 1) * SUB],
                                         w2T[fc], start=(fc == 0), stop=last)
                    if use_b2row:
                        nc.tensor.matmul(pd, ones_row[:, PAD + t0:PAD + t0 + SUB],
                                         crows[:, 5 * C:6 * C], start=False, stop=True)
                    ot = outp.tile([128, C], dt.float32, tag="o")
                    nc.vector.tensor_add(out=ot, in0=x1_tiles[k], in1=pd)
                    nc.sync.dma_start(out=out_d[r, t0:t0 + SUB, :], in_=ot)

    nc.compile()
    return nc


_CACHE = {}


def _get_nc(key):
    if key not in _CACHE:
        _CACHE[key] = _build(use_crows=key[0], use_b2row=key[1])
    return _CACHE[key]


def _run(inputs, trace_dir=None):
    from concourse.bass_utils import run_bass_kernel_spmd
    from concourse import bass2jax

    x = np.asarray(inputs["x"], dtype=np.float32)
    w, use_crows, use_b2row = _prep(inputs)
    nc = _get_nc((use_crows, use_b2row))

    in_maps = []
    for core in range(NCORES):
        m = dict(w)
        m["x"] = np.ascontiguousarray(x[core * BPC:(core + 1) * BPC])
        in_maps.append(m)

    if trace_dir is None:
        res = run_bass_kernel_spmd(nc, in_maps, list(range(NCORES)))
        results, exec_ns = res.results, None
    else:
        from antenv.axon_hooks import get_axon_ntff_profile_hook

        hook = get_axon_ntff_profile_hook()
        os.makedirs(trace_dir, exist_ok=True)
        with hook(trace_dir, [0]):
            results = bass2jax.run_bass_via_pjrt(nc, in_maps, n_cores=NCORES)
        exec_ns = None  # caller post-processes the NTFFs

    out = np.concatenate([np.asarray(results[i]["out"]) for i in range(NCORES)], axis=0)
    return out, exec_ns


def kernel(**inputs):
    out, _ = _run(inputs)
    return out


# revision 4
# speedup vs baseline: 1.6515x; 1.6515x over previous
"""Trainium2 Bass kernel for nn_Block_627065225827 (dense_transformer).

Self-contained: hardcodes shapes B=32, T=4096, C=256, H=8 and the
data-parallel-over-batch sharding (4 batch rows per core, 8 cores).

Math (see reference):
    h   = LN1(x) * g1 + b1ln
    id  = h @ w_id.T ;  inf = h @ w_inf.T            (per-head view [H, hs])
    inf = inf / (1+K);  shifted[t] = inf[t - s_h]    (zero for t < s_h)
    sa  = (K/(1+K) * id + shifted) @ w_proj.T + b_proj
    x1  = x + sa
    ff  = relu(LN2(x1)*g2+b2ln @ w1.T + b1) @ w2.T + b2
    out = x1 + ff

Host-side algebraic folding (exact):
    sa[t] = w_a @ xhat[t] + sum_s W_s @ xhat[t-s] + const_rows
      w_a = w_proj @ (diag(a_row) @ (w_id * g1))           a_h = K/(1+K)
      W_s = w_proj[:, cols_s] @ ((w_inf * g1) * binv)[cols_s, :]
    so the per-head temporal shift becomes a free-dim offset into the
    transposed activation buffer hB (channels on partitions, tokens on
    free dim), with 4 zero pad columns implementing the t<s mask.
"""

import os
from contextlib import ExitStack

import numpy as np
import ml_dtypes

B, T, C, H = 32, 4096, 256, 8
HS = C // H
NCORES = 8
BPC = B // NCORES  # batch rows per core
SHIFTS = [1, 2, 3, 4, 1, 2, 3, 4]
EPS = 1e-5
PAD = 4  # zero columns at the head of hB for the shift mask
WIN = 512  # tokens per window
SUB = 128  # tokens per subtile (partition dim)

_f64 = np.float64
_bf16 = ml_dtypes.bfloat16


def _prep(inputs):
    """Fold LN gains/biases + per-head scalars into the weights (host, numpy)."""
    g = {k: np.asarray(v, dtype=_f64) for k, v in inputs.items() if k != "x"}
    K = np.exp(g["khead"])  # [H]
    a_row = np.repeat(K / (1.0 + K), HS)  # [C] per id-output channel
    b_row = np.repeat(1.0 / (1.0 + K), HS)  # [C] per inf-output channel

    w_id_g = g["w_id"] * g["ln1_g"][None, :]
    w_inf_g = g["w_inf"] * g["ln1_g"][None, :]
    w_id_s = w_id_g * a_row[:, None]
    w_inf_s = w_inf_g * b_row[:, None]

    w_a = g["w_proj"] @ w_id_s  # [C, C]
    wsT = np.zeros((4, C, C), _f64)
    c_s = np.zeros((4, C), _f64)
    cid = w_id_g @ g["ln1_b"]
    cinf = w_inf_g @ g["ln1_b"]
    for s in range(1, 5):
        cols = np.concatenate(
            [np.arange(h * HS, (h + 1) * HS) for h in range(H) if SHIFTS[h] == s]
        )
        wsT[s - 1] = (g["w_proj"][:, cols] @ w_inf_s[cols, :]).T
        c_s[s - 1] = g["w_proj"][:, cols] @ (b_row * cinf)[cols]
    c_a = g["w_proj"] @ (a_row * cid) + g["b_proj"]  # [C]

    w1_g = g["w1"] * g["ln2_g"][None, :]
    b1_eff = g["w1"] @ g["ln2_b"] + g["b1"]  # [4C]
    w2T = g["w2"].T  # [4C, C]
    b2_eff = g["b2"]  # [C]

    out = {
        "w_aT": np.ascontiguousarray(w_a.T.reshape(2, 128, C)).astype(_bf16),
        "wsT": np.ascontiguousarray(wsT.reshape(4, 2, 128, C)).astype(_bf16),
        "w1T": np.ascontiguousarray(w1_g.T.reshape(2, 128, 4 * C)).astype(_bf16),
        "w2T": np.ascontiguousarray(w2T.reshape(8, 128, C)).astype(_bf16),
        "b1c": np.ascontiguousarray(b1_eff.reshape(8, 128).T).astype(np.float32),
    }
    crows_attn = np.stack([c_a, c_s[0], c_s[1], c_s[2], c_s[3]])  # [5, C]
    use_crows = bool(np.abs(crows_attn).max() > 0)
    use_b2row = bool(np.abs(b2_eff).max() > 0)
    if use_crows or use_b2row:
        out["crows"] = np.concatenate([crows_attn, b2_eff[None, :]]).reshape(1, 6 * C).astype(_bf16)
    return out, use_crows, use_b2row


def _build(n_rows=BPC, t_len=T, use_crows=False, use_b2row=False):
    """Build + compile the per-core Bass program. Returns the finalized nc."""
    import concourse.bacc as bacc
    import concourse.mybir as mybir
    import concourse.tile as tile
    from concourse.masks import make_identity

    dt = mybir.dt
    AF = mybir.ActivationFunctionType
    OP = mybir.AluOpType

    nwin = t_len // WIN
    nc = bacc.Bacc("TRN2", target_bir_lowering=False, debug=False, num_devices=NCORES)

    x_d = nc.declare_dram_parameter("x", [n_rows, t_len, C], dt.float32, isOutput=False)
    waT_d = nc.declare_dram_parameter("w_aT", [2, 128, C], dt.bfloat16, isOutput=False)
    wsT_d = nc.declare_dram_parameter("wsT", [4, 2, 128, C], dt.bfloat16, isOutput=False)
    w1T_d = nc.declare_dram_parameter("w1T", [2, 128, 4 * C], dt.bfloat16, isOutput=False)
    w2T_d = nc.declare_dram_parameter("w2T", [8, 128, C], dt.bfloat16, isOutput=False)
    b1c_d = nc.declare_dram_parameter("b1c", [128, 8], dt.float32, isOutput=False)
    crows_d = None
    if use_crows or use_b2row:
        crows_d = nc.declare_dram_parameter("crows", [1, 6 * C], dt.bfloat16, isOutput=False)
    out_d = nc.declare_dram_parameter("out", [n_rows, t_len, C], dt.float32, isOutput=True)

    with tile.TileContext(nc) as tc, ExitStack() as ctx:
        singles = ctx.enter_context(tc.tile_pool(name="singles", bufs=1))
        hb_pool = ctx.enter_context(tc.tile_pool(name="hb", bufs=1))
        xin = ctx.enter_context(tc.tile_pool(name="xin", bufs=12))
        x1p = ctx.enter_context(tc.tile_pool(name="x1p", bufs=12))
        hnorm = ctx.enter_context(tc.tile_pool(name="hnorm", bufs=8))
        stats = ctx.enter_context(tc.tile_pool(name="stats", bufs=16))
        h2b = ctx.enter_context(tc.tile_pool(name="h2b", bufs=2))
        ffb = ctx.enter_context(tc.tile_pool(name="ffb", bufs=2))
        outp = ctx.enter_context(tc.tile_pool(name="outp", bufs=8))
        tp_ps = ctx.enter_context(tc.tile_pool(name="tp_ps", bufs=2, space="PSUM"))
        sa_ps = ctx.enter_context(tc.tile_pool(name="sa_ps", bufs=2, space="PSUM"))
        up_ps = ctx.enter_context(tc.tile_pool(name="up_ps", bufs=2, space="PSUM"))
        dn_ps = ctx.enter_context(tc.tile_pool(name="dn_ps", bufs=2, space="PSUM"))

        # ---- constants / weights in SBUF ----
        ident = singles.tile([128, 128], dt.bfloat16)
        make_identity(nc, ident)
        eps_t = singles.tile([128, 1], dt.float32)
        nc.vector.memset(eps_t, EPS)
        waT = []
        for c in range(2):
            w = singles.tile([128, C], dt.bfloat16, tag=f"waT{c}")
            nc.gpsimd.dma_start(out=w, in_=waT_d[c])
            waT.append(w)
        wsT = []
        for s in range(4):
            row = []
            for c in range(2):
                w = singles.tile([128, C], dt.bfloat16, tag=f"wsT{s}{c}")
                nc.gpsimd.dma_start(out=w, in_=wsT_d[s, c])
                row.append(w)
            wsT.append(row)
        w1T = []
        for c in range(2):
            w = singles.tile([128, 4 * C], dt.bfloat16, tag=f"w1T{c}")
            nc.gpsimd.dma_start(out=w, in_=w1T_d[c])
            w1T.append(w)
        w2T = []
        for fc in range(8):
            w = singles.tile([128, C], dt.bfloat16, tag=f"w2T{fc}")
            nc.gpsimd.dma_start(out=w, in_=w2T_d[fc])
            w2T.append(w)
        b1c = singles.tile([128, 8], dt.float32)
        nc.gpsimd.dma_start(out=b1c, in_=b1c_d[:, :])
        crows = None
        ones_row = None
        if crows_d is not None:
            crows = singles.tile([1, 6 * C], dt.bfloat16)
            nc.gpsimd.dma_start(out=crows, in_=crows_d[:, :])
            ones_row = singles.tile([1, PAD + t_len], dt.bfloat16)
            nc.vector.memset(ones_row, 1.0)
            nc.vector.memset(ones_row[:, 0:PAD], 0.0)

        def layernorm_to(src, dst0, dst1):
            """LN(src [128,C] f32) -> bf16, transposed into dst0/dst1 [128,128]."""
            st = stats.tile([128, 6], dt.float32, tag="st")
            nc.vector.bn_stats(st, src)
            mv = stats.tile([128, 2], dt.float32, tag="mv")
            nc.vector.bn_aggr(mv, st)
            sd = stats.tile([128, 1], dt.float32, tag="sd")
            nc.scalar.activation(sd, mv[:, 1:2], AF.Sqrt, bias=eps_t, scale=1.0)
            rs = stats.tile([128, 1], dt.float32, tag="rs")
            nc.vector.reciprocal(rs, sd)
            hn = hnorm.tile([128, C], dt.bfloat16, tag="hn")
            nc.vector.tensor_scalar(
                out=hn, in0=src, scalar1=mv[:, 0:1], scalar2=rs,
                op0=OP.subtract, op1=OP.mult,
            )
            for c, dst in enumerate((dst0, dst1)):
                pt = tp_ps.tile([128, 128], dt.bfloat16, tag="tp")
                nc.tensor.transpose(pt, hn[:, 128 * c:128 * (c + 1)], ident)
                if c == 0:
                    nc.vector.tensor_copy(out=dst, in_=pt)
                else:
                    nc.scalar.copy(out=dst, in_=pt)

        def window_body(slot, hB, r, w):
            t0w = w * WIN
            x_tiles, x1_tiles = [], []
            h2 = [h2b.tile([128, WIN], dt.bfloat16, tag=f"h2b{slot}{c}",
                           name=f"h2b{slot}{c}") for c in range(2)]
            for k in range(4):
                t0 = t0w + k * SUB
                col = PAD + t0
                xt = xin.tile([128, C], dt.float32, tag="x")
                nc.sync.dma_start(out=xt, in_=x_d[r, t0:t0 + SUB, :])
                x_tiles.append(xt)
                layernorm_to(xt, hB[0][:, col:col + SUB], hB[1][:, col:col + SUB])
            for k in range(4):
                t0 = t0w + k * SUB
                col = PAD + t0
                # attention: sa[t, co] accumulated over 10 matmuls
                ps = sa_ps.tile([128, C], dt.float32, tag="sa")
                nc.tensor.matmul(ps, hB[0][:, col:col + SUB], waT[0],
                                 start=True, stop=False)
                nc.tensor.matmul(ps, hB[1][:, col:col + SUB], waT[1],
                                 start=False, stop=False)
                for s in range(1, 5):
                    for c in range(2):
                        last = (s == 4 and c == 1 and not use_crows)
                        nc.tensor.matmul(
                            ps, hB[c][:, col - s:col - s + SUB], wsT[s - 1][c],
                            start=False, stop=last,
                        )
                if use_crows:
                    nc.tensor.matmul(ps, ones_row[:, col:col + SUB], crows[:, 0:C],
                                     start=False, stop=False)
                    for s in range(1, 5):
                        nc.tensor.matmul(
                            ps, ones_row[:, col - s:col - s + SUB],
                            crows[:, s * C:(s + 1) * C], start=False, stop=(s == 4),
                        )
                x1t = x1p.tile([128, C], dt.float32, tag="x1")
                nc.vector.tensor_add(out=x1t, in0=x_tiles[k], in1=ps)
                x1_tiles.append(x1t)
            for k in range(4):
                layernorm_to(x1_tiles[k], h2[0][:, k * SUB:(k + 1) * SUB],
                             h2[1][:, k * SUB:(k + 1) * SUB])

            # FFN up + relu (weights stationary, tokens moving)
            fftiles = []
            for fc in range(8):
                pu = up_ps.tile([128, WIN], dt.float32, tag="up")
                nc.tensor.matmul(pu, w1T[0][:, 128 * fc:128 * (fc + 1)], h2[0],
                                 start=True, stop=False)
                nc.tensor.matmul(pu, w1T[1][:, 128 * fc:128 * (fc + 1)], h2[1],
                                 start=False, stop=True)
                fb = ffb.tile([128, WIN], dt.bfloat16, tag=f"ffb{slot}{fc}",
                              name=f"ffb{slot}{fc}")
                nc.scalar.activation(fb, pu, AF.Relu, bias=b1c[:, fc:fc + 1],
                                     scale=1.0)
                fftiles.append(fb)

            # FFN down (ff stationary, w2T moving) + residual + store
            for k in range(4):
                t0 = t0w + k * SUB
                pd = dn_ps.tile([128, C], dt.float32, tag="dn")
                for fc in range(8):
                    last = (fc == 7 and not use_b2row)
                    nc.tensor.matmul(pd, fftiles[fc][:, k * SUB:(k + 1) * SUB],
                                     w2T[fc], start=(fc == 0), stop=last)
                if use_b2row:
                    nc.tensor.matmul(pd, ones_row[:, PAD + t0:PAD + t0 + SUB],
                                     crows[:, 5 * C:6 * C], start=False, stop=True)
                ot = outp.tile([128, C], dt.float32, tag="o")
                nc.vector.tensor_add(out=ot, in0=x1_tiles[k], in1=pd)
                nc.sync.dma_start(out=out_d[r, t0:t0 + SUB, :], in_=ot)

        # Interleave pairs of batch rows: while one row's LN chains run on
        # DVE/ACT, the other row's matmuls keep the PE dense (HAM warm).
        nslots = min(2, n_rows)
        for rp in range(0, n_rows, nslots):
            hBs = []
            for slot in range(nslots):
                hB = []
                for c in range(2):
                    t_ = hb_pool.tile([128, PAD + t_len], dt.bfloat16,
                                      tag=f"hb{slot}{c}", name=f"hb{slot}{c}")
                    nc.gpsimd.memset(t_[:, 0:PAD], 0.0)
                    hB.append(t_)
                hBs.append(hB)
            for w in range(nwin):
                for slot in range(nslots):
                    window_body(slot, hBs[slot], rp + slot, w)

    nc.compile()
    return nc


_CACHE = {}


def _get_nc(key):
    if key not in _CACHE:
        _CACHE[key] = _build(use_crows=key[0], use_b2row=key[1])
    return _CACHE[key]


def _run(inputs, trace_dir=None):
    from concourse.bass_utils import run_bass_kernel_spmd
    from concourse import bass2jax

    x = np.asarray(inputs["x"], dtype=np.float32)
    w, use_crows, use_b2row = _prep(inputs)
    nc = _get_nc((use_crows, use_b2row))

    in_maps = []
    for core in range(NCORES):
        m = dict(w)
        m["x"] = np.ascontiguousarray(x[core * BPC:(core + 1) * BPC])
        in_maps.append(m)

    if trace_dir is None:
        res = run_bass_kernel_spmd(nc, in_maps, list(range(NCORES)))
        results, exec_ns = res.results, None
    else:
        from antenv.axon_hooks import get_axon_ntff_profile_hook

        hook = get_axon_ntff_profile_hook()
        os.makedirs(trace_dir, exist_ok=True)
        with hook(trace_dir, [0]):
            results = bass2jax.run_bass_via_pjrt(nc, in_maps, n_cores=NCORES)
        exec_ns = None  # caller post-processes the NTFFs

    out = np.concatenate([np.asarray(results[i]["out"]) for i in range(NCORES)], axis=0)
    return out, exec_ns


def kernel(**inputs):
    out, _ = _run(inputs)
    return out
